# revision 25
# baseline (speedup 1.0000x reference)
"""GQA causal self-attention with ALiBi — Trainium2 Bass kernel, 8 NeuronCores.

Sharding: one (batch, kv-head) pair per core (2 batches x 4 kv heads = 8 cores).
Each core computes its 4 query heads' attention over the full sequence and a
partial output projection y_partial = att_heads @ Wo[head_rows]; the host sums
the 4 partials per batch in f32.

Device-side math (per core, T=2048, HD=64, G=4 query heads, slope s):
  QKV^T = (x @ [Wq_g*scale, Wk_g, Wv_g])^T          (x^T pre-transposed on host)
  S^T[j,i] = q_i . k_j * scale - (s*i + SHIFT)      (shift row via matmul aug row)
  P^T = exp(S^T + s*j)                              (s*j = per-partition ACT bias)
  P^T masked causally (affine_select zero-fill on the diagonal blocks)
  [attT_unnorm | l] = [V | 1]^T augmented PV matmul (l lands on PSUM partition 64)
  attT = attT_unnorm * (1/l broadcast via K=1 ones-row matmul from partition 64)
  y = attT^T @ Wo_rows                              (attT is lhsT directly)

Matmul inputs are bf16 (PSUM accumulation stays f32): same PE rate as fp32r
but half the DMA / SBUF / DVE traffic, and no N>=256 restriction so the
causal skip can drop 3 of 4 masked 128-blocks on diagonal chunks.  The
per-query shift -(s*i+SHIFT) (including its bf16 rounding error — a
per-query factor) cancels exactly between attT_unnorm and l; s*j enters
through the f32 ACT bias exactly.

Scheduling: phase C's interior is ACT(exp)-bound while PE has slack, so the
span-1 QKV projection (during C(0)) and the first half of the output
projection (during C(1)) are chopped into sub-microsecond pieces and emitted
one per key-block iteration instead of as head-boundary bursts.  DMAs:
SP/Pool/ACT each own an independent queue that a transfer occupies fully;
x tiles are striped over ACT+Pool, weights and thin alibi rows ride SP, and
y stores ride SP (the scheduler reorders ready DMAs, so assignments — not
emission order — carry the intent).
"""

import math
import numpy as np

import concourse.bass as bass
import concourse.mybir as mybir
import concourse.tile as tile
from concourse import bacc
from concourse.bass_utils import run_bass_kernel_spmd

f32 = mybir.dt.float32
f32r = mybir.dt.float32r
bf16 = mybir.dt.bfloat16
EXP = mybir.ActivationFunctionType.Exp

B, T, C = 2, 2048, 1024
H, HKV, HD = 16, 4, 64
G = H // HKV              # 4 query heads per core
GH = G * HD               # 256
QKV = GH + 2 * HD         # 384 projection cols per core
SCALE = 1.0 / math.sqrt(HD)
SHIFT = 4.0
NKT = T // 128            # 16 key blocks of 128
NQC = T // 512            # 4 query chunks of 512

_CACHED_NC = None


def _build_nc(reps=1):
    nc = bacc.Bacc("TRN2", target_bir_lowering=False, debug=False)

    xT = nc.dram_tensor("xT", [C, T], bf16, kind="ExternalInput")
    wqkv = nc.dram_tensor("wqkv", [C, QKV], bf16, kind="ExternalInput")
    wo = nc.dram_tensor("wo", [GH, C], bf16, kind="ExternalInput")
    aux = nc.dram_tensor("aux", [4, T], bf16, kind="ExternalInput")
    sjcol = nc.dram_tensor("sjcol", [128, NKT], f32, kind="ExternalInput")
    y = nc.dram_tensor("y", [T, C], bf16, kind="ExternalOutput")

    with tile.TileContext(nc) as tc:
        for r in range(reps):
            _emit(nc, tc, xT, wqkv, wo, aux, sjcol, y, sfx=f"_{r}" if r else "")

    nc.finalize()
    return nc


def _emit(nc, tc, xT, wqkv, wo, aux, sjcol, y, sfx=""):
    import contextlib
    ctx = contextlib.ExitStack()
    with ctx:
        const = ctx.enter_context(tc.tile_pool(name="const" + sfx, bufs=1))
        xpool = ctx.enter_context(tc.tile_pool(name="xpool" + sfx, bufs=24))
        ptpool = ctx.enter_context(tc.tile_pool(name="ptpool" + sfx, bufs=6))
        vtpool = ctx.enter_context(tc.tile_pool(name="vtpool" + sfx, bufs=2))
        ypool = ctx.enter_context(tc.tile_pool(name="ypool" + sfx, bufs=4))
        lpool = ctx.enter_context(tc.tile_pool(name="lpool" + sfx, bufs=2))
        # PSUM (8 banks): psbig 2x2-bank S^T supertiles, psos 2x1-bank PV
        # accumulators (both live across a whole head), pssm 2x1-bank
        # transients (QKV pb / transposes / 1-l broadcasts / y projections).
        psbig = ctx.enter_context(tc.tile_pool(name="psbig" + sfx, bufs=2, space="PSUM"))
        psos = ctx.enter_context(tc.tile_pool(name="psos" + sfx, bufs=2, space="PSUM"))
        pssm = ctx.enter_context(tc.tile_pool(name="pssm" + sfx, bufs=2, space="PSUM"))

        # ---- constants / persistent tensors ----
        wqkv_sb = const.tile([128, C // 128, QKV], bf16, name="wqkv_sb")
        wqkv_r = wqkv.rearrange("(o p) m -> p o m", p=128)
        for c8 in range(8):
            nc.sync.dma_start(wqkv_sb[:, c8, :], wqkv_r[:, c8, :])
        sj_sb = const.tile([128, NKT], f32, name="sj_sb")
        nc.sync.dma_start(sj_sb, sjcol[:, :])

        # 65 = 64 k/q features + one augmentation row: kaug row 64 is all
        # ones (memset — a 1-partition DMA is charged per free byte), qaug
        # row 64 is -(s*i + SHIFT) so the kaug-ones x qaug-negm product
        # applies the per-query stabilizing shift inside the S^T matmul.
        KA = 65
        kaug = const.tile([KA, T], bf16, name="kaug")
        nc.vector.memset(kaug[64:65, :], 1.0)
        qaug = [const.tile([KA, T], bf16, name=f"qaug{h}") for h in range(G)]
        for h in range(G):
            nc.sync.dma_start(qaug[h][64:65, :], aux[2:3, :])  # negm
        wo_sb = const.tile([128, GH // 128, C], bf16, name="wo_sb")
        nc.sync.dma_start(wo_sb, wo.rearrange("(o p) n -> p o n", p=128))

        # [V | ones] layout: PV-matmul column HD is the all-ones row, so the
        # softmax denominator l lands on PSUM partition 64 — a legal aligned
        # partition base for the reciprocal and the K=1 broadcast rhs.
        v_sb = const.tile([128, NKT, HD + 1], bf16, name="v_sb")
        for kt in range(NKT):
            nc.vector.memset(v_sb[:, kt, HD:HD + 1], 1.0)

        att = [const.tile([128, T], bf16, name=f"att{c}") for c in range(2)]

        # 1/l broadcast operands: lrecs[h] row 64 receives the reciprocal
        # (partition 64 -> 64, no cross-partition move).  Two K=65 matmuls
        # accumulate the even/odd head's row into the low/high 64 partitions
        # of one full-height PSUM tile (matmul outputs must start at
        # partition 0) via 0/1 selection lhsTs — same ISA shapes as S^T/PV.
        sel = []
        for par in range(2):
            se = const.tile([65, 128], f32r, name=f"sel{par}")
            nc.vector.memset(se.bitcast(f32), 0.0)
            nc.vector.memset(se[64:65, 64 * par:64 * par + 64].bitcast(f32), 1.0)
            sel.append(se)
        lrecs = []
        for h in range(G):
            lr = const.tile([65, T], f32r, name=f"lrec{h}")
            nc.vector.memset(lr[0:64, :].bitcast(f32), 0.0)
            lrecs.append(lr)

        ident_f = const.tile([64, 64], f32, name="ident_f")
        nc.gpsimd.memset(ident_f, 0.0)
        nc.gpsimd.affine_select(
            out=ident_f, in_=ident_f, compare_op=mybir.AluOpType.not_equal,
            fill=1.0, base=0, pattern=[[-1, 64]], channel_multiplier=1)
        ident = const.tile([64, 64], bf16, name="ident")
        nc.vector.tensor_copy(ident, ident_f)

        # ---- x loads, striped across the ACT and Pool queues ----
        _xts = {0: [[None] * 8 for _ in range(2)], 1: [[None] * 8 for _ in range(2)]}

        def load_xt(tc2, engs):
            tcol = tc2 * 1024
            i = 0
            for nn in range(2):
                for c8 in range(8):
                    xt = xpool.tile([128, 512], bf16, name=f"xt{tc2}_{nn}_{c8}", tag="xt")
                    engs[i % len(engs)].dma_start(
                        xt, xT[c8 * 128:(c8 + 1) * 128,
                               tcol + nn * 512:tcol + (nn + 1) * 512])
                    i += 1
                    _xts[tc2][nn][c8] = xt

        load_xt(0, [nc.scalar, nc.gpsimd])
        load_xt(1, [nc.gpsimd])

        # causal mask: zero P^T where j > i, i.e. keep iff q - p - 128*r >= 0
        # (q = query idx within 512-chunk, p = key idx within block, r = block
        # offset within the chunk); applied as gpsimd affine_select on the
        # exp output (values pass through / fill 0.0).  Only the [off:512)
        # region is touched — the skipped prefix is never written by exp nor
        # read by the PV matmul.
        def causal_mask(pt, off, r):
            nc.gpsimd.affine_select(
                out=pt[:, off:512], in_=pt[:, off:512],
                compare_op=mybir.AluOpType.is_ge, fill=0.0,
                base=off - 128 * r, pattern=[[1, 512 - off]],
                channel_multiplier=-1)

        # ---- phase B: QKV^T projection, as a generator of small pieces ----
        # Each piece is <=4 accumulation matmuls (~0.4us of PE) so pieces can
        # slot into phase C's ACT-bound kt loop without starving the exps.
        def b_pieces(tc2, mts=(2, 0, 1)):
            tcol = tc2 * 1024
            xts = _xts[tc2]
            for mt in mts:
                for nn in range(2):
                    pcol = tcol + nn * 512
                    pb = pssm.tile([128, 512], f32, name=f"pqkv{tc2}_{mt}_{nn}", tag="sm")

                    def mm(c8s, pb=pb, mt=mt, nn=nn):
                        for c8 in c8s:
                            nc.tensor.matmul(
                                pb,
                                lhsT=wqkv_sb[:, c8, mt * 128:(mt + 1) * 128],
                                rhs=xts[nn][c8],
                                start=(c8 == 0), stop=(c8 == 7))
                    yield lambda: mm(range(0, 4))
                    yield lambda: mm(range(4, 8))

                    if mt < 2:
                        def evq(pb=pb, mt=mt, pcol=pcol):
                            nc.vector.tensor_copy(
                                qaug[2 * mt][0:64, pcol:pcol + 512], pb[0:64, :])
                            nc.vector.tensor_copy(
                                qaug[2 * mt + 1][0:64, pcol:pcol + 512], pb[64:128, :])
                        yield evq
                    else:
                        vt = vtpool.tile([64, 512], bf16, name=f"vt{tc2}_{nn}", tag="vt")

                        def evk(pb=pb, vt=vt, pcol=pcol):
                            nc.vector.tensor_copy(kaug[0:64, pcol:pcol + 512], pb[0:64, :])
                            nc.vector.tensor_copy(vt, pb[64:128, :])
                        yield evk

                        def tr(i2, vt=vt, tc2=tc2, nn=nn):
                            for i in i2:
                                pt_ps = pssm.tile([128, 64], bf16, name=f"ptr{tc2}_{nn}_{i}", tag="sm")
                                nc.tensor.transpose(pt_ps, vt[:, i * 128:(i + 1) * 128], ident)
                                nc.vector.tensor_copy(
                                    v_sb[:, tc2 * 8 + nn * 4 + i, 0:HD], pt_ps)
                        yield lambda: tr((0, 1))
                        yield lambda: tr((2, 3))

        def run_all(gen):
            for piece in gen:
                piece()

        # ---- phase D: output projection, as pieces (one per qt half) ----
        def d_pieces(qts):
            for qt in qts:
                ysb = ypool.tile([128, C], bf16, name=f"ysb{qt}", tag="ysb")
                for n2 in range(2):
                    def dj(qt=qt, n2=n2, ysb=ysb):
                        yp = pssm.tile([128, 512], f32, name=f"yp{qt}_{n2}", tag="sm")
                        for c2 in range(2):
                            nc.tensor.matmul(yp,
                                             lhsT=att[c2][:, qt * 128:(qt + 1) * 128],
                                             rhs=wo_sb[:, c2, n2 * 512:(n2 + 1) * 512],
                                             start=(c2 == 0), stop=(c2 == 1))
                        nc.vector.tensor_copy(ysb[:, n2 * 512:(n2 + 1) * 512], yp)
                    yield dj

                def st(qt=qt, ysb=ysb):
                    nc.sync.dma_start(y[qt * 128:(qt + 1) * 128, :], ysb)
                yield st

        # ---- phase C: attention, key-block-major within one 1024-query group
        # fill: iterator of filler pieces; skip_heads: heads whose kt loops
        # get no fillers (their inputs may still be in flight early in C(0)).
        def emit_c(qcg, fill=None, skip_heads=()):
            fill = iter(fill) if fill is not None else None

            def put_filler():
                if fill is None:
                    return
                piece = next(fill, None)
                if piece is not None:
                    piece()

            ls_pair = [None, None]
            for h in range(G):
                qa, qb = 2 * qcg, 2 * qcg + 1      # the two 512-query chunks
                osum_a = psos.tile([HD + 1, 512], f32, name=f"osa{qcg}_{h}", tag="os")
                osum_b = psos.tile([HD + 1, 512], f32, name=f"osb{qcg}_{h}", tag="os")
                ka_last = 4 * qa + 3               # last key block for chunk a
                kb_last = 4 * qb + 3
                for kt in range(kb_last + 1):
                    sp = psbig.tile([128, 1024], f32, name=f"sp{qcg}_{h}_{kt}", tag="big")
                    pt = ptpool.tile([128, 1024], bf16, name=f"pt{qcg}_{h}_{kt}", tag="pt")
                    if kt <= ka_last:
                        # both chunks attend this key block. For the chunk
                        # containing the diagonal, queries below the block are
                        # fully masked: skip them in matmul+exp (the mask pass
                        # zero-fills the skipped prefix).
                        ra = kt - 4 * qa
                        off = min(ra, 3) * 128 if ra > 0 else 0
                        nc.tensor.matmul(sp[:, off:512], lhsT=kaug[:, kt * 128:(kt + 1) * 128],
                                         rhs=qaug[h][:, qa * 512 + off:(qa + 1) * 512],
                                         start=True, stop=True)
                        nc.tensor.matmul(sp[:, 512:1024], lhsT=kaug[:, kt * 128:(kt + 1) * 128],
                                         rhs=qaug[h][:, qb * 512:(qb + 1) * 512],
                                         start=True, stop=True)
                        nc.scalar.activation(pt[:, off:1024], sp[:, off:1024], EXP,
                                             bias=sj_sb[:, kt:kt + 1])
                        if ra >= 0:
                            causal_mask(pt, off, ra)
                        nc.tensor.matmul(osum_a[:, off:512], lhsT=v_sb[:, kt, :],
                                         rhs=pt[:, off:512],
                                         start=(kt == 0), stop=(kt == ka_last))
                        nc.tensor.matmul(osum_b, lhsT=v_sb[:, kt, :], rhs=pt[:, 512:1024],
                                         start=(kt == 0), stop=(kt == kb_last))
                    else:
                        # only chunk b attends; always causally partial
                        rb = kt - 4 * qb
                        off = min(rb, 3) * 128
                        nc.tensor.matmul(sp[:, off:512], lhsT=kaug[:, kt * 128:(kt + 1) * 128],
                                         rhs=qaug[h][:, qb * 512 + off:(qb + 1) * 512],
                                         start=True, stop=True)
                        nc.scalar.activation(pt[:, off:512], sp[:, off:512], EXP,
                                             bias=sj_sb[:, kt:kt + 1])
                        causal_mask(pt, off, rb)
                        nc.tensor.matmul(osum_b[:, off:512], lhsT=v_sb[:, kt, :],
                                         rhs=pt[:, off:512],
                                         start=False, stop=(kt == kb_last))
                    if h not in skip_heads:
                        put_filler()
                # evacuate: att rows (osum partitions 0..64) + per-head 1/l
                # (reciprocal straight off PSUM partition 64)
                c2, half = h // 2, (h % 2) * 64
                nc.vector.tensor_copy(att[c2][half:half + 64, qa * 512:(qa + 1) * 512],
                                      osum_a[0:HD, :])
                nc.vector.tensor_copy(att[c2][half:half + 64, qb * 512:(qb + 1) * 512],
                                      osum_b[0:HD, :])
                with nc.allow_low_precision(reason="softmax reciprocal to fp32r"):
                    nc.vector.reciprocal(lrecs[h][64:65, qa * 512:(qa + 1) * 512],
                                         osum_a[HD:HD + 1, :])
                    nc.vector.reciprocal(lrecs[h][64:65, qb * 512:(qb + 1) * 512],
                                         osum_b[HD:HD + 1, :])
                ls_pair[h % 2] = lrecs[h]
                if h % 2 == 1:
                    # both heads of att chunk c2 done: normalize it now so the
                    # output projection can start without waiting for all heads.
                    # 1/l rows are broadcast partition-64 -> 64 partitions by
                    # a K=1 matmul against a ones row (full rate, N=512).
                    c2n = h // 2
                    for qi, qc in enumerate((2 * qcg, 2 * qcg + 1)):
                        rp = pssm.tile([128, 512], f32, name=f"rp{qcg}_{c2n}_{qc}", tag="sm")
                        qc0 = qc * 512
                        nc.tensor.matmul(rp, lhsT=sel[0],
                                         rhs=ls_pair[0][:, qc0:qc0 + 512],
                                         start=True, stop=False)
                        nc.tensor.matmul(rp, lhsT=sel[1],
                                         rhs=ls_pair[1][:, qc0:qc0 + 512],
                                         start=False, stop=True)
                        nc.vector.tensor_tensor(att[c2n][:, qc * 512:(qc + 1) * 512],
                                                att[c2n][:, qc * 512:(qc + 1) * 512], rp,
                                                mybir.AluOpType.mult)

        run_all(b_pieces(0))
        # span-1 QKV pieces slot into C(0)'s kt loops (h=0 skipped: the
        # span-1 x tiles may still be in flight that early)
        emit_c(0, fill=b_pieces(1), skip_heads=(0,))
        # first-half output projection slots into C(1)'s kt loops
        emit_c(1, fill=d_pieces(range(8)))
        run_all(d_pieces(range(8, 16)))


def _alibi_slopes(n_heads):
    start = 2.0 ** (-(2.0 ** (-(math.log2(n_heads) - 3))))
    return np.array([start * (start ** i) for i in range(n_heads)], dtype=np.float32)


def kernel(x, Wq, Wk, Wv, Wo):
    global _CACHED_NC
    if _CACHED_NC is None:
        _CACHED_NC = _build_nc()
    nc = _CACHED_NC

    np_bf16 = mybir.dt.np(bf16)
    x = np.asarray(x, dtype=np.float32)
    Wq = np.asarray(Wq, dtype=np.float32)
    Wk = np.asarray(Wk, dtype=np.float32)
    Wv = np.asarray(Wv, dtype=np.float32)
    Wo = np.asarray(Wo, dtype=np.float32)

    slopes = _alibi_slopes(H)[:HKV]
    ar = np.arange(T, dtype=np.float32)

    in_maps = []
    for b in range(B):
        xT_b = np.ascontiguousarray(x[b].T.astype(np_bf16))
        for g in range(HKV):
            s = float(slopes[g])
            wq_g = Wq[:, g * GH:(g + 1) * GH] * SCALE
            wk_g = Wk[:, g * HD:(g + 1) * HD]
            wv_g = Wv[:, g * HD:(g + 1) * HD]
            wqkv_m = np.ascontiguousarray(
                np.concatenate([wq_g, wk_g, wv_g], axis=1).astype(np_bf16))
            wo_g = np.ascontiguousarray(Wo[g * GH:(g + 1) * GH, :].astype(np_bf16))
            negm = -(s * ar + SHIFT)
            aux = np.ascontiguousarray(
                np.stack([np.ones(T, np.float32), np.zeros(T, np.float32),
                          negm, np.ones(T, np.float32)]).astype(np_bf16))
            sjcol = np.ascontiguousarray((s * ar).reshape(NKT, 128).T)
            in_maps.append({
                "xT": xT_b, "wqkv": wqkv_m, "wo": wo_g,
                "aux": aux, "sjcol": sjcol,
            })

    global _last_in_maps
    _last_in_maps = in_maps
    res = run_bass_kernel_spmd(nc, in_maps, list(range(B * HKV)))
    out = np.zeros((B, T, C), dtype=np.float32)
    for b in range(B):
        for g in range(HKV):
            out[b] += np.asarray(res.results[b * HKV + g]["y"],
                                 dtype=np.float32)
    return out
